# revision 12
# baseline (speedup 1.0000x reference)
"""Trainium2 Bass kernel for nn_Encoder_51900384804901.

6-layer post-norm TransformerEncoder (E=1024, NH=16, DFF=4096, relu FFN)
where every token attends only to the first num_ctx=1024 context tokens.

Sharding: data-parallel over batch. B=8 -> one batch element per NeuronCore,
no collectives. Each core runs the full 6-layer encoder on its [2048, 1024]
slice with activations resident in SBUF (feature-major x^T [E, T]) and
weights streamed from HBM. All GEMMs run in float32r (full PE rate, ~1e-5
relative error).

Per layer:
  phase 1: K^T [E,NC] feature-major + V' token-major (V plus a ones column
           per head, so the PV matmul also produces the softmax denominator)
  phase 2 (per 512-token q-chunk): Q^T, scores S^T = K^T.T @ Q^T per head,
           exp on ScalarE (1/sqrt(dh) folded into the activation scale; no
           max-subtraction needed -- scores are bounded by construction),
           PV matmul, reciprocal + partition_broadcast + one DVE multiply,
           out-projection accumulated into the residual
  phase 3 (per q-chunk): LN1, FFN1(relu), FFN2, LN2 -- LayerNorm reductions
           over the partition axis via ones-matmuls.

Self-contained: hardcodes all shapes; host pre-transposes weights.
"""

import numpy as np

import concourse.bacc as bacc
import concourse.tile as tile
from concourse import mybir
from concourse import bass_utils

# Problem dims (hardcoded per contract)
L, E, NH, DFF = 6, 1024, 16, 4096
B, S, NC = 8, 2048, 1024
DH = E // NH  # 64
LN_EPS = 1e-5

F32 = mybir.dt.float32
F32R = mybir.dt.float32r

P = 128          # partitions
NQ = S // 512    # 4 q-chunks of 512
NCC = NC // 512  # 2 ctx chunks of 512
ET = E // P      # 8 e-tiles
FT = DFF // P    # 32 f-tiles
KT = NC // P     # 8 ctx k-tiles


def r32(ap):
    return ap.bitcast(F32R)


def f32(ap):
    return ap.bitcast(F32)


def build_encoder():
    nc = bacc.Bacc("TRN2", debug=False)

    xT = nc.dram_tensor("xT", [E, S], F32R, kind="ExternalInput").ap()
    wqkvT = nc.dram_tensor("wqkvT", [L, E, 3 * E], F32R, kind="ExternalInput").ap()
    woT = nc.dram_tensor("woT", [L, E, E], F32R, kind="ExternalInput").ap()
    w1T = nc.dram_tensor("w1T", [L, E, DFF], F32R, kind="ExternalInput").ap()
    w2T = nc.dram_tensor("w2T", [L, DFF, E], F32R, kind="ExternalInput").ap()
    # params[l]: [128, 104] per-partition param columns:
    # 0-7 bq | 8-15 bk | 16-23 bv | 24-31 bo | 32-63 b1 | 64-71 b2
    # 72-79 g1 | 80-87 be1 | 88-95 g2 | 96-103 be2   (col m <-> e-tile m)
    params = nc.dram_tensor("params", [L, P, 104], F32, kind="ExternalInput").ap()
    bvrow = nc.dram_tensor("bvrow", [L, E], F32, kind="ExternalInput").ap()
    onesd = nc.dram_tensor("onesd", [P, 16], F32R, kind="ExternalInput").ap()
    outT = nc.dram_tensor("outT", [E, S], F32R, kind="ExternalOutput").ap()

    AF = mybir.ActivationFunctionType
    OP = mybir.AluOpType

    with tile.TileContext(nc) as tc:
        with tc.tile_pool(name="persist", bufs=1) as pp, \
             tc.tile_pool(name="psum", bufs=8, space="PSUM") as psp:
            # Residual stream x^T, resident for the whole kernel
            xt = [pp.tile([P, S], F32R, tag=f"x{i}", name=f"x{i}") for i in range(ET)]
            for i in range(ET):
                nc.sync.dma_start(xt[i][:], xT[P * i:P * (i + 1), :])
            ones = pp.tile([P, 1], F32R, name="ones")
            nc.sync.dma_start(ones[:], onesd[:, 0:1])
            eps_t = pp.tile([1, 1], F32, tag="eps", name="eps")
            nc.vector.memset(eps_t[:], LN_EPS)

            def ps_tile():
                return psp.tile([P, 512], F32, tag="ps", name="ps")

            def gemm_512(wdram, mg, rhs_tiles, nk, wpool, wtag, evict):
                """out[m] = sum_k W[k, mg*512+mi*128 ...].T @ rhs[k] for the 4
                m-subtiles of one 512-wide weight chunk column group."""
                ps4 = [ps_tile() for _ in range(4)]
                for k in range(nk):
                    w = wpool.tile([P, 512], F32R, tag=wtag, name=wtag)
                    nc.sync.dma_start(
                        w[:], wdram[P * k:P * (k + 1),
                                    mg * 512:(mg + 1) * 512])
                    for mi in range(4):
                        nc.tensor.matmul(
                            ps4[mi][:],
                            r32(w[:, P * mi:P * (mi + 1)]),
                            r32(rhs_tiles[k]),
                            start=(k == 0), stop=(k == nk - 1))
                for mi in range(4):
                    evict(mg * 4 + mi, ps4[mi])

            def layernorm(qc, g_col, b_col, par, sq_pool, ln_pool):
                """x[:, qc] = LN(x[:, qc]) * g + b, in place (feature axis)."""
                cs = slice(qc * 512, (qc + 1) * 512)
                # sum(x) over features via ones-matmul
                s1 = ps_tile()
                for k in range(ET):
                    nc.tensor.matmul(s1[0:1, :], r32(ones[:]), r32(xt[k][:, cs]),
                                     start=(k == 0), stop=(k == ET - 1))
                # sum(x^2)
                s2 = ps_tile()
                for k in range(ET):
                    sq = sq_pool.tile([P, 512], F32R, tag="sq", name="sq", bufs=2)
                    nc.scalar.activation(sq[:], f32(xt[k][:, cs]), AF.Square)
                    nc.tensor.matmul(s2[0:1, :], r32(ones[:]), r32(sq[:]),
                                     start=(k == 0), stop=(k == ET - 1))
                mean = ln_pool.tile([1, 512], F32, tag="mean", name="mean", bufs=2)
                nc.vector.tensor_scalar_mul(mean[:], s1[0:1, :], 1.0 / E)
                msq = ln_pool.tile([1, 512], F32, tag="msq", name="msq", bufs=2)
                nc.scalar.activation(msq[:], mean[:], AF.Square)
                var = ln_pool.tile([1, 512], F32, tag="var", name="var", bufs=2)
                nc.vector.tensor_scalar_mul(var[:], s2[0:1, :], 1.0 / E)
                nc.vector.tensor_tensor(var[:], var[:], msq[:], OP.subtract)
                # rstd = 1/sqrt(var + eps)
                nc.scalar.activation(var[:], var[:], AF.Sqrt, bias=eps_t[:])
                rstd = ln_pool.tile([1, 512], F32, tag="rstd", name="rstd", bufs=2)
                nc.vector.reciprocal(rstd[:], var[:])
                nmr = ln_pool.tile([1, 512], F32, tag="nmr", name="nmr", bufs=2)
                nc.vector.tensor_tensor(nmr[:], mean[:], rstd[:], OP.mult)
                nc.vector.tensor_scalar_mul(nmr[:], nmr[:], -1.0)
                ab = ln_pool.tile([P, 512], F32, tag="ab", name="ab", bufs=2)
                nc.gpsimd.partition_broadcast(ab[:], rstd[:])
                bb = ln_pool.tile([P, 512], F32, tag="bb", name="bb", bufs=2)
                nc.gpsimd.partition_broadcast(bb[:], nmr[:])
                for k in range(ET):
                    t1 = ln_pool.tile([P, 512], F32, tag="t1", name="t1", bufs=2)
                    nc.vector.tensor_tensor(t1[:], f32(xt[k][:, cs]), ab[:], OP.mult)
                    nc.vector.tensor_tensor(t1[:], t1[:], bb[:], OP.add)
                    nc.vector.tensor_scalar(
                        xt[k][:, cs], t1[:],
                        par[:, g_col + k:g_col + k + 1],
                        par[:, b_col + k:b_col + k + 1],
                        OP.mult, OP.add)

            for l in range(L):
                with tc.tile_pool(name="par", bufs=1) as parp:
                    par = parp.tile([P, 104], F32, tag="par", name="par")
                    nc.sync.dma_start(par[:], params[l])

                    # ---- Phase 1+2: K/V then attention over all q-chunks ----
                    with tc.tile_pool(name="kv", bufs=1) as kvp:
                        kt = [kvp.tile([P, NC], F32R, tag=f"k{i}", name=f"k{i}")
                              for i in range(ET)]
                        vp = [kvp.tile([P, NH * (DH + 1)], F32R, tag=f"v{i}", name=f"v{i}")
                              for i in range(KT)]
                        with tc.tile_pool(name="kvw", bufs=1) as kvw:
                            bvb = kvw.tile([P, E], F32, tag="bvb", name="bvb")
                            bvr = kvw.tile([1, E], F32, tag="bvr", name="bvr")
                            nc.sync.dma_start(bvr[:], bvrow[l][None, :])
                            nc.gpsimd.partition_broadcast(bvb[:], bvr[:])
                            # K projection (feature-major out)
                            wk = [kvw.tile([P, E], F32R, tag=f"wk{k}", name=f"wk{k}")
                                  for k in range(ET)]
                            for k in range(ET):
                                nc.sync.dma_start(
                                    wk[k][:], wqkvT[l, P * k:P * (k + 1), E:2 * E])
                            for m in range(ET):
                                for cc in range(NCC):
                                    ps = ps_tile()
                                    for k in range(ET):
                                        nc.tensor.matmul(
                                            ps[:],
                                            r32(wk[k][:, P * m:P * (m + 1)]),
                                            r32(xt[k][:, cc * 512:(cc + 1) * 512]),
                                            start=(k == 0), stop=(k == ET - 1))
                                    nc.scalar.activation(
                                        kt[m][:, cc * 512:(cc + 1) * 512], ps[:],
                                        AF.Identity, bias=par[:, 8 + m:9 + m])
                            # V projection (token-major out), x_ctx as lhsT
                            wv = [kvw.tile([P, E], F32R, tag=f"wk{k}", name=f"wk{k}")
                                  for k in range(ET)]
                            for k in range(ET):
                                nc.sync.dma_start(
                                    wv[k][:], wqkvT[l, P * k:P * (k + 1),
                                                    2 * E:3 * E])
                            for t in range(KT):
                                for ch in range(2):
                                    ps = ps_tile()
                                    for k in range(ET):
                                        nc.tensor.matmul(
                                            ps[:],
                                            r32(xt[k][:, P * t:P * (t + 1)]),
                                            r32(wv[k][:, ch * 512:(ch + 1) * 512]),
                                            start=(k == 0), stop=(k == ET - 1))
                                    for hh in range(8):
                                        h = ch * 8 + hh
                                        nc.vector.tensor_tensor(
                                            vp[t][:, h * 65:h * 65 + 64],
                                            ps[:, hh * 64:(hh + 1) * 64],
                                            bvb[:, h * 64:(h + 1) * 64], OP.add)
                                ones_cols = vp[t].rearrange(
                                    "p (h c) -> p h c", c=DH + 1)[:, :, 64:65]
                                nc.sync.dma_start(
                                    ones_cols, onesd[:, :, None])

                        # Attention + out-projection per q-chunk
                        with tc.tile_pool(name="aw", bufs=4) as aw, \
                             tc.tile_pool(name="att", bufs=1) as att, \
                             tc.tile_pool(name="es", bufs=8) as esp, \
                             tc.tile_pool(name="sc", bufs=3) as scp:
                            for qc in range(NQ):
                                cs = slice(qc * 512, (qc + 1) * 512)
                                qt = [att.tile([P, 512], F32R, tag=f"q{i}", name=f"q{i}")
                                      for i in range(ET)]
                                rhs_x = [xt[k][:, cs] for k in range(ET)]

                                def ev_q(m, ps):
                                    nc.scalar.activation(
                                        qt[m][:], ps[:], AF.Identity,
                                        bias=par[:, m:m + 1])
                                for mg in range(2):
                                    gemm_512(wqkvT[l], mg, rhs_x, ET, aw, "w",
                                             ev_q)
                                # attention per head
                                at = [att.tile([P, 512], F32R, tag=f"a{i}", name=f"a{i}")
                                      for i in range(ET)]
                                for h in range(NH):
                                    p_, off = h // 2, (h % 2) * 64
                                    es_tiles = []
                                    for k in range(KT):
                                        ps = ps_tile()
                                        nc.tensor.matmul(
                                            ps[:],
                                            r32(kt[p_][off:off + 64,
                                                       P * k:P * (k + 1)]),
                                            r32(qt[p_][off:off + 64, :]),
                                            start=True, stop=True)
                                        es = esp.tile([P, 512], F32R, tag="es", name="es")
                                        nc.scalar.activation(
                                            es[:], ps[:], AF.Exp,
                                            scale=float(1.0 / np.sqrt(DH)))
                                        es_tiles.append(es)
                                    po = ps_tile()
                                    for k in range(KT):
                                        nc.tensor.matmul(
                                            po[0:DH + 1, :],
                                            r32(vp[k][:, h * 65:(h + 1) * 65]),
                                            r32(es_tiles[k][:]),
                                            start=(k == 0), stop=(k == KT - 1))
                                    rc = scp.tile([1, 512], F32, tag="rc",
                                                  name="rc", bufs=2)
                                    nc.vector.reciprocal(rc[:], po[DH:DH + 1, :])
                                    bct = scp.tile([64, 512], F32, tag="bct",
                                                   name="bct", bufs=2)
                                    nc.gpsimd.partition_broadcast(bct[:], rc[:])
                                    nc.vector.tensor_tensor(
                                        at[p_][off:off + 64, :], po[0:DH, :],
                                        bct[:], OP.mult)

                                # out-projection, residual add into xt
                                def ev_o(m, ps):
                                    tmp = scp.tile([P, 512], F32, tag="tmp", name="tmp")
                                    nc.scalar.activation(
                                        tmp[:], ps[:], AF.Identity,
                                        bias=par[:, 24 + m:25 + m])
                                    nc.vector.tensor_tensor(
                                        xt[m][:, cs], f32(xt[m][:, cs]), tmp[:],
                                        OP.add)
                                rhs_a = [at[k][:] for k in range(ET)]
                                for mg in range(2):
                                    gemm_512(woT[l], mg, rhs_a, ET, aw, "w",
                                             ev_o)

                    # ---- Phase 3: LN1 + FFN + LN2 per q-chunk ----
                    with tc.tile_pool(name="ffw", bufs=4) as ffw, \
                         tc.tile_pool(name="hp", bufs=1) as hp, \
                         tc.tile_pool(name="sc2", bufs=3) as sc2:
                        for qc in range(NQ):
                            cs = slice(qc * 512, (qc + 1) * 512)
                            layernorm(qc, 72, 80, par, sc2, sc2)
                            ht = [hp.tile([P, 512], F32R, tag=f"h{i}", name=f"h{i}")
                                  for i in range(FT)]
                            rhs_x = [xt[k][:, cs] for k in range(ET)]

                            def ev_h(m, ps):
                                nc.scalar.activation(
                                    ht[m][:], ps[:], AF.Relu,
                                    bias=par[:, 32 + m:33 + m])
                            for mg in range(8):
                                gemm_512(w1T[l], mg, rhs_x, ET, ffw, "w", ev_h)

                            def ev_f2(m, ps):
                                tmp = sc2.tile([P, 512], F32, tag="tmp", name="tmp")
                                nc.scalar.activation(
                                    tmp[:], ps[:], AF.Identity,
                                    bias=par[:, 64 + m:65 + m])
                                nc.vector.tensor_tensor(
                                    xt[m][:, cs], f32(xt[m][:, cs]), tmp[:],
                                    OP.add)
                            rhs_h = [ht[k][:] for k in range(FT)]
                            for mg in range(2):
                                gemm_512(w2T[l], mg, rhs_h, FT, ffw, "w2",
                                         ev_f2)
                            layernorm(qc, 88, 96, par, sc2, sc2)

            for i in range(ET):
                nc.sync.dma_start(outT[P * i:P * (i + 1), :], xt[i][:])

    nc.compile()
    return nc


def _prep_inputs(inputs):
    """Host-side: transpose weights / pack params; returns per-core in_maps."""
    emb = np.asarray(inputs["embeddings"], dtype=np.float32)
    ipw = np.asarray(inputs["in_proj_w"], dtype=np.float32)   # [L, 3E, E]
    ipb = np.asarray(inputs["in_proj_b"], dtype=np.float32)   # [L, 3E]
    ow = np.asarray(inputs["out_w"], dtype=np.float32)        # [L, E, E]
    ob = np.asarray(inputs["out_b"], dtype=np.float32)        # [L, E]
    l1w = np.asarray(inputs["lin1_w"], dtype=np.float32)      # [L, DFF, E]
    l1b = np.asarray(inputs["lin1_b"], dtype=np.float32)      # [L, DFF]
    l2w = np.asarray(inputs["lin2_w"], dtype=np.float32)      # [L, E, DFF]
    l2b = np.asarray(inputs["lin2_b"], dtype=np.float32)      # [L, E]
    g1 = np.asarray(inputs["ln1_w"], dtype=np.float32)
    be1 = np.asarray(inputs["ln1_b"], dtype=np.float32)
    g2 = np.asarray(inputs["ln2_w"], dtype=np.float32)
    be2 = np.asarray(inputs["ln2_b"], dtype=np.float32)

    wqkvT = np.ascontiguousarray(ipw.transpose(0, 2, 1))      # [L, E, 3E]
    woT = np.ascontiguousarray(ow.transpose(0, 2, 1))         # [L, E, E]
    w1T = np.ascontiguousarray(l1w.transpose(0, 2, 1))        # [L, E, DFF]
    w2T = np.ascontiguousarray(l2w.transpose(0, 2, 1))        # [L, DFF, E]

    def cols(a, n):  # [L, n*128] -> [L, 128, n]
        return a.reshape(L, n, P).transpose(0, 2, 1)

    params = np.concatenate([
        cols(ipb[:, 0:E], 8), cols(ipb[:, E:2 * E], 8), cols(ipb[:, 2 * E:], 8),
        cols(ob, 8), cols(l1b, 32), cols(l2b, 8),
        cols(g1, 8), cols(be1, 8), cols(g2, 8), cols(be2, 8),
    ], axis=2)
    params = np.ascontiguousarray(params, dtype=np.float32)   # [L, 128, 104]
    bvrow = np.ascontiguousarray(ipb[:, 2 * E:3 * E])         # [L, E]

    shared = dict(wqkvT=wqkvT, woT=woT, w1T=w1T, w2T=w2T,
                  params=params, bvrow=bvrow,
                  onesd=np.ones((P, 16), np.float32))
    in_maps = []
    for c in range(B):
        m = dict(shared)
        m["xT"] = np.ascontiguousarray(emb[c].T)              # [E, S]
        in_maps.append(m)
    return in_maps


_NC_CACHE = {}


def _get_nc():
    if "nc" not in _NC_CACHE:
        _NC_CACHE["nc"] = build_encoder()
    return _NC_CACHE["nc"]


def run(inputs, trace=False, tmpdir=None):
    """Run on 8 NeuronCores; returns (output [8, S, E], BassKernelResults)."""
    in_maps = _prep_inputs(inputs)
    nc = _get_nc()
    res = bass_utils.run_bass_kernel_spmd(
        nc, in_maps, core_ids=list(range(B)), trace=trace, tmpdir=tmpdir)
    out = np.stack([np.ascontiguousarray(res.results[c]["outT"].T)
                    for c in range(B)])
    return out, res


def kernel(**inputs):
    num_ctx = int(np.asarray(inputs["num_ctx"]))
    assert num_ctx == NC, f"kernel hardcodes num_ctx={NC}, got {num_ctx}"
    out, _ = run(inputs)
    return out
